# revision 2
# baseline (speedup 1.0000x reference)
"""CrossModalAttention TRN2 kernel v6: PE-offloaded reductions, 4-slab pipe.

Changes vs v5 (197.8us, wrong results):
 - GpSimd dropped entirely: its tensor_copy mishandled the stride-0
   broadcast AP (wrong e1 values) AND any GP activity port-blocks
   concurrent DVE tensor_tensor ops down to 1x.  All e1 expansion on ACT.
 - k1 (the 1-element-shifted K copy) no longer a 23us SBUF->SBUF DMA
   that landed 35us late: DVE now builds it chunk-by-chunk right after
   each K evacuation (shifted bf16 copy, 1x due to odd dst offset).
 - Projection evacuation split ACT (low half) / DVE scalar_tensor_tensor
   (high half, different PSUM banks): the chunk cadence drops from
   3.4us (ACT-paced + HAM-cold PE) to ~1.7us and the PE duty cycle gets
   high enough to hold K=8/8.

Changes vs v4 (kernel.py baseline, 196.7us):
 - QK tree-reduce (47us DVE) replaced by PE identity-matmul PSUM
   accumulation over the 16 token pixels (16 mms x 128 cols per
   neighbor).  PE holds ONE stationary identity weight through the whole
   attention phase (no Ldweights churn) and never idles long enough to
   re-throttle (HAM stays at K=8/8).
 - Mask becomes an ADDITIVE -30000 bias accumulated as a 17th matmul
   into the score PSUM; exp underflows masked slots to exactly 0 (the
   separate em = e*mask DVE multiply disappears).
 - Z = sum_n e via 9 more identity matmuls (PSUM accumulate), and
   1/Z = exp(-ln Z) on ACT (Exp+Ln+Identity+Copy live in one table
   set) -- the expensive DVE reciprocal disappears.
 - AV multiply fused 4->1 per neighbor via a stride-0 u-broadcast AP on
   the attention weights (innermost stays stride-1 so DVE keeps 2x).
 - Work split into 4 slabs (2 channel-halves x 2 token-row-halves) so
   score PSUM (3 banks) + z (1 bank) + AV accumulator (4 banks) fit the
   8 PSUM banks, and the slab phases interleave across DVE/PE/ACT/GP.
 - e1 (attention-weight v-expansion) alternates GpSimd / ACT.
"""

import os
from contextlib import ExitStack

import numpy as np

import concourse.bass as bass
import concourse.mybir as mybir
import concourse.tile as tile
from concourse.bass_utils import run_bass_kernel_spmd

B, C, H, W = 2, 256, 128, 128
TOK = 4
NH, NW = H // TOK, W // TOK          # 32, 32
T2 = TOK * TOK
SCALE = float((C // T2) ** -0.5)
NCORES = 8
QH = 4
NH_LOC = NH // QH                    # 8 token rows / core
ROWSZ = TOK * W                      # 512 px per token row
SLAB = NH_LOC * ROWSZ                # 4096 px per channel-half
HALO_ROWS = NH_LOC + 2
PIX_HALO = HALO_ROWS * ROWSZ         # 5120
NIJ = NH_LOC * NW                    # 256 tokens per core
OFFS = [(di, dj) for di in (-1, 0, 1) for dj in (-1, 0, 1)]
NN = len(OFFS)                       # 9
SNN = NN * NIJ                       # 2304 score slots per channel-half

# slab = (channel-half, token-row-half): 4 token rows each
IH = NH_LOC // 2                     # 4 token rows per slab
HSLAB = IH * ROWSZ                   # 2048 px per slab
HNIJ = IH * NW                       # 128 tokens per slab
HSNN = NN * HNIJ                     # 1152 score slots per slab
MASKV = -30000.0                     # additive mask: exp(s-30000) == 0

F32 = mybir.dt.float32
FP16 = mybir.dt.float16
BF16 = mybir.dt.bfloat16
AF = mybir.ActivationFunctionType

KPAD = 2                             # even pad: dj=0 reads stay 4B-aligned
VPAD = TOK                           # pixel layout: dj shift = +-4 elems
KSZ = KPAD + PIX_HALO + KPAD
K1SZ = PIX_HALO + 2                  # K shifted by one elem: dj=+-1 aligned
VSZ = VPAD + PIX_HALO + VPAD


def _build_kernel(nc: bass.Bass, ctx: ExitStack, tc: "tile.TileContext"):
    xb = nc.dram_tensor("xb", [C, SLAB], FP16, kind="ExternalInput").ap()
    xw = nc.dram_tensor("xw", [C, PIX_HALO], FP16, kind="ExternalInput").ap()
    wq = nc.dram_tensor("wq", [C, C], FP16, kind="ExternalInput").ap()
    wk = nc.dram_tensor("wk", [C, C], FP16, kind="ExternalInput").ap()
    wv = nc.dram_tensor("wv", [C, C], FP16, kind="ExternalInput").ap()
    bq = nc.dram_tensor("bq", [2, 128, 1], F32, kind="ExternalInput").ap()
    bk = nc.dram_tensor("bk", [2, 128, 1], F32, kind="ExternalInput").ap()
    bv = nc.dram_tensor("bv", [2, 128, 1], F32, kind="ExternalInput").ap()
    mask = nc.dram_tensor("mask", [128, SNN], BF16, kind="ExternalInput").ap()
    ident = nc.dram_tensor("ident", [128, 128], BF16,
                           kind="ExternalInput").ap()
    out = nc.dram_tensor("out", [C, SLAB], BF16, kind="ExternalOutput").ap()

    const_pool = ctx.enter_context(tc.tile_pool(name="const", bufs=1))

    # ---- persistent constants
    w_sb = {}
    for name, wd in (("q", wq), ("k", wk), ("v", wv)):
        for ci in range(2):
            t = const_pool.tile([128, C], FP16, tag=f"w{name}{ci}",
                                name=f"w{name}{ci}")
            nc.sync.dma_start(t[:], wd[ci * 128:(ci + 1) * 128, :])
            w_sb[name, ci] = t
    b_sb = {}
    for name, bd in (("q", bq), ("k", bk), ("v", bv)):
        for co in range(2):
            t = const_pool.tile([128, 1], F32, tag=f"b{name}{co}",
                                name=f"b{name}{co}")
            nc.sync.dma_start(t[:], bd[co])
            b_sb[name, co] = t
    mask_sb = const_pool.tile([128, SNN], BF16, tag="mask", name="mask")
    nc.sync.dma_start(mask_sb[:], mask[:])
    id_sb = const_pool.tile([128, 128], BF16, tag="ident", name="ident")
    nc.sync.dma_start(id_sb[:], ident[:])

    # ACT/DVE warm-ups: cover the bias/mask DMAs on their consumer engines
    # so the real consumers carry at most one sync wait (walrus limit).
    scratch = const_pool.tile([128, 16], F32, tag="scratch", name="scratch")
    for wi, name in enumerate(("q", "k", "v")):
        for co in range(2):
            nc.scalar.activation(scratch[:, wi * 2 + co:wi * 2 + co + 1],
                                 b_sb[name, co][:], AF.Identity,
                                 bias=b_sb[name, co][:])
            # DVE also consumes the biases (STT proj evacs) -- warm them
            nc.vector.tensor_copy(scratch[:, 8 + wi * 2 + co:
                                          9 + wi * 2 + co],
                                  b_sb[name, co][:])
    nc.vector.tensor_copy(scratch[:, 6:7], mask_sb[:, 0:1])

    # attention SBUF pools (allocated before x so x can be scoped/freed)
    qkv_pool = ctx.enter_context(tc.tile_pool(name="qkv", bufs=1))
    e_pool = ctx.enter_context(tc.tile_pool(name="e", bufs=2))
    an_pool = ctx.enter_context(tc.tile_pool(name="an", bufs=2))
    ln_pool = ctx.enter_context(tc.tile_pool(name="ln", bufs=2))
    zr_pool = ctx.enter_context(tc.tile_pool(name="zr", bufs=2))
    prod_pool = ctx.enter_context(tc.tile_pool(name="prod", bufs=3))
    e1_pool = ctx.enter_context(tc.tile_pool(name="e1", bufs=3))
    avt_pool = ctx.enter_context(tc.tile_pool(name="avt", bufs=3))
    of_pool = ctx.enter_context(tc.tile_pool(name="of", bufs=2))

    q_sb = [qkv_pool.tile([128, SLAB], BF16, tag=f"q{c}", name=f"q{c}")
            for c in range(2)]
    k_sb = [qkv_pool.tile([128, KSZ], BF16, tag=f"k{c}", name=f"k{c}")
            for c in range(2)]
    k1_sb = [qkv_pool.tile([128, K1SZ], BF16, tag=f"k1{c}", name=f"k1{c}")
             for c in range(2)]
    v_sb = [qkv_pool.tile([128, VSZ], BF16, tag=f"v{c}", name=f"v{c}")
            for c in range(2)]
    for t in k_sb:
        nc.vector.memset(t[:, 0:KPAD], 0.0)
        nc.vector.memset(t[:, KPAD + PIX_HALO:], 0.0)
    for t in k1_sb:
        # chunked k1 copies cover y in [1, 5121); the two edge elements
        # mirror k_sb pad zeros
        nc.vector.memset(t[:, 0:1], 0.0)
        nc.vector.memset(t[:, K1SZ - 1:], 0.0)
    for t in v_sb:
        nc.vector.memset(t[:, 0:VPAD], 0.0)
        nc.vector.memset(t[:, VPAD + PIX_HALO:], 0.0)

    with tc.tile_pool(name="ps", bufs=2, space="PSUM") as ps_pool:
        # PE warm-up: cover weight/ident/mask DMAs on PE's clock.
        warm_ps = ps_pool.tile([128, 2048], F32, tag="ps", name="warm")
        for name in ("q", "k", "v"):
            for ci in range(2):
                nc.tensor.matmul(warm_ps[0:1, 0:1], w_sb[name, ci][:, 0:1],
                                 w_sb[name, ci][:, 0:1],
                                 start=True, stop=True)
        nc.tensor.matmul(warm_ps[0:1, 0:1], id_sb[:, 0:1], id_sb[:, 0:1],
                         start=True, stop=True)
        nc.tensor.matmul(warm_ps[0:1, 0:1], id_sb[:, 0:1], mask_sb[:, 0:1],
                         start=True, stop=True)

        # ---- projections (PE matmul, evacuation via ACT)
        with tc.tile_pool(name="x", bufs=1) as x_pool:
            xb_sb = [x_pool.tile([128, SLAB], FP16, tag=f"xb{ci}",
                                 name=f"xb{ci}") for ci in range(2)]
            xw_sb = [x_pool.tile([128, PIX_HALO], FP16, tag=f"xw{ci}",
                                 name=f"xw{ci}") for ci in range(2)]
            for ci in range(2):
                rows = slice(ci * 128, (ci + 1) * 128)
                for c0 in range(0, SLAB, 1024):
                    nc.sync.dma_start(xb_sb[ci][:, c0:c0 + 1024],
                                      xb[rows, c0:c0 + 1024])
            for ci in range(2):
                rows = slice(ci * 128, (ci + 1) * 128)
                for c0 in range(0, PIX_HALO, 1024):
                    nc.sync.dma_start(xw_sb[ci][:, c0:c0 + 1024],
                                      xw[rows, c0:c0 + 1024])
                # dummy matmuls put every x-DMA queue on PE's clock
                for pt in range(SLAB // 512):
                    nc.tensor.matmul(warm_ps[:, 0:64],
                                     xb_sb[ci][:, pt * 512:pt * 512 + 128],
                                     xb_sb[ci][:, pt * 512:pt * 512 + 64],
                                     start=True, stop=True)
                for pt in range(PIX_HALO // 512):
                    nc.tensor.matmul(warm_ps[:, 0:64],
                                     xw_sb[ci][:, pt * 512:pt * 512 + 128],
                                     xw_sb[ci][:, pt * 512:pt * 512 + 64],
                                     start=True, stop=True)

            def project(name, co, src, dst, pix, pad, scale, token_order,
                        k1dst=None):
                for c0 in range(0, pix, 2048):
                    cw = min(2048, pix - c0)
                    ps = ps_pool.tile([128, 2048], F32, tag="ps")
                    for ci in range(2):
                        for s0 in range(0, cw, 512):
                            sl = slice(c0 + s0, c0 + s0 + 512)
                            rhs = src[ci][:, sl]
                            if token_order:
                                rhs = rhs.rearrange("p (u j v) -> p u v j",
                                                    u=TOK, j=NW, v=TOK)
                            nc.tensor.matmul(
                                ps[:, s0:s0 + 512],
                                w_sb[name, ci][:, co * 128:(co + 1) * 128],
                                rhs, start=(ci == 0), stop=(ci == 1))
                    if k1dst is not None:
                        # full ACT evac, then DVE builds the shifted K1
                        # copy of this chunk (k1[y] = k[y+1])
                        nc.scalar.activation(
                            dst[co][:, pad + c0:pad + c0 + cw],
                            ps[:, 0:cw], AF.Identity,
                            bias=b_sb[name, co][:], scale=scale)
                        nc.vector.tensor_copy(
                            k1dst[co][:, c0 + 1:c0 + 1 + cw],
                            dst[co][:, c0 + 2:c0 + 2 + cw])
                    else:
                        # split evac: ACT low half (banks 0-1), DVE STT
                        # high half (banks 2-3) -- keeps the chunk cadence
                        # at the PE's pace so HAM stays warm
                        h1 = cw // 2
                        nc.scalar.activation(
                            dst[co][:, pad + c0:pad + c0 + h1],
                            ps[:, 0:h1], AF.Identity,
                            bias=b_sb[name, co][:], scale=scale)
                        nc.vector.scalar_tensor_tensor(
                            dst[co][:, pad + c0 + h1:pad + c0 + cw],
                            ps[:, h1:cw], float(scale),
                            b_sb[name, co][:].broadcast_to((128, cw - h1)),
                            mybir.AluOpType.mult, mybir.AluOpType.add)

            # channel-half 0 first so attention slabs 0/1 start early
            for co in range(2):
                project("q", co, xb_sb, q_sb, SLAB, 0, SCALE, True)
                # K shifted one element left: dj=+-1 products read K1 at
                # even (4B-aligned) offsets, keeping DVE 2x
                project("k", co, xw_sb, k_sb, PIX_HALO, KPAD, 1.0, True,
                        k1dst=k1_sb)
                project("v", co, xw_sb, v_sb, PIX_HALO, VPAD, 1.0, False)

    # ---- attention PSUM pools (after proj pool closed): 3 + 1 + 4 banks
    s_pool = ctx.enter_context(tc.tile_pool(name="sps", bufs=1,
                                            space="PSUM"))
    z_pool = ctx.enter_context(tc.tile_pool(name="zps", bufs=1,
                                            space="PSUM"))
    av_pool = ctx.enter_context(tc.tile_pool(name="avps", bufs=1,
                                             space="PSUM"))

    mask_v = mask_sb[:].rearrange("p (n i j) -> p n i j", n=NN, i=NH_LOC)
    s_ts = {}
    e_ts = {}
    an_ts = {}
    accs = {}

    def qk_phase(t):
        ch, hh = divmod(t, 2)
        # [128, 1536] f32 = 3 banks; scores live in [0:1152]
        s_t = s_pool.tile([128, 3 * 512], F32, tag="s", name=f"s{t}")
        s_ts[t] = s_t
        q_sl = q_sb[ch][:, hh * HSLAB:(hh + 1) * HSLAB]
        for n in (1, 4, 7, 0, 2, 3, 5, 6, 8):
            di, dj = OFFS[n]
            base = (hh * IH + di + 1) * ROWSZ
            if dj == 0:
                ksrc, koff = k_sb[ch], KPAD + base
            else:
                # K1[y] = K[y+1] -> K[base + dj + t] = K1[base + dj + 1 + t]
                ksrc, koff = k1_sb[ch], base + dj + 1
            prod = prod_pool.tile([128, HSLAB], BF16, tag="prod",
                                  name="prod")
            nc.vector.tensor_mul(prod[:], q_sl,
                                 ksrc[:, koff:koff + HSLAB])
            # PE: sum over the 16 token pixels (u,v) + additive mask bias
            pv = prod[:].rearrange("p (i u v j) -> p u v i j",
                                   i=IH, u=TOK, v=TOK)
            sreg = s_t[:, n * HNIJ:(n + 1) * HNIJ]
            for u in range(TOK):
                for v in range(TOK):
                    nc.tensor.matmul(sreg, id_sb[:], pv[:, u, v],
                                     start=(u == 0 and v == 0), stop=False)
            mv = mask_v[:, n, hh * IH:(hh + 1) * IH]
            nc.tensor.matmul(sreg, id_sb[:], mv, start=False, stop=True)

    def sm_a_phase(t):
        # everything that touches the score PSUM tile: exp, Z-accum, ln
        s_t = s_ts[t]
        e_t = e_pool.tile([128, HSNN], BF16, tag="e", name=f"e{t}")
        e_ts[t] = e_t
        nc.scalar.activation(e_t[:], s_t[:, 0:HSNN], AF.Exp)
        z_t = z_pool.tile([128, 512], F32, tag="z", name=f"z{t}")
        ev = e_t[:].rearrange("p (n ij) -> p n ij", n=NN)
        for n in range(NN):
            nc.tensor.matmul(z_t[:, 0:HNIJ], id_sb[:], ev[:, n],
                             start=(n == 0), stop=(n == NN - 1))
        lnz = ln_pool.tile([128, HNIJ], F32, tag="lnz", name=f"lnz{t}")
        nc.scalar.activation(lnz[:], z_t[:, 0:HNIJ], AF.Ln)
        s_ts[t] = lnz

    def sm_b_phase(t):
        lnz = s_ts[t]
        zrb = zr_pool.tile([128, HNIJ], BF16, tag="zrb", name=f"zrb{t}")
        nc.scalar.activation(zrb[:], lnz[:], AF.Exp, scale=-1.0)
        an_t = an_pool.tile([128, HSNN], BF16, tag="an", name=f"an{t}")
        ev = e_ts[t][:].rearrange("p (n ij) -> p n ij", n=NN)
        anv = an_t[:].rearrange("p (n ij) -> p n ij", n=NN)
        zb = zrb[:].unsqueeze(1).broadcast_to((128, NN, HNIJ))
        nc.vector.tensor_mul(anv, ev, zb)
        an_ts[t] = an_t

    def av_phase(t):
        ch, hh = divmod(t, 2)
        an_t = an_ts[t]
        acc = av_pool.tile([128, HSLAB], F32, tag="av", name=f"acc{t}")
        accs[t] = acc
        for n, (di, dj) in enumerate(OFFS):
            # v-expansion of the attention weights (x4), GP/ACT alternating
            e1 = e1_pool.tile([128, HNIJ * TOK], BF16, tag="e1", name="e1")
            an_n = an_t[:, n * HNIJ:(n + 1) * HNIJ].rearrange(
                "p (i j) -> p i j", i=IH)
            e1v = e1[:].rearrange("p (i j v) -> p i j v", i=IH, j=NW)
            src = an_n.unsqueeze(3).broadcast_to((128, IH, NW, TOK))
            nc.scalar.copy(e1v, src)
            # fused AV multiply: u-broadcast via stride-0 mid-dim (DVE 2x)
            voff = VPAD + (hh * IH + 1 + di) * ROWSZ + dj * TOK
            avt = avt_pool.tile([128, HSLAB], BF16, tag="avt", name="avt")
            av_v = avt[:].rearrange("p (i u jv) -> p i u jv", i=IH, u=TOK)
            e1b = e1[:].rearrange("p (i jv) -> p i jv", i=IH) \
                .unsqueeze(2).broadcast_to((128, IH, TOK, TOK * NW))
            vv = v_sb[ch][:, voff:voff + HSLAB].rearrange(
                "p (i u jv) -> p i u jv", i=IH, u=TOK)
            nc.vector.tensor_mul(av_v, e1b, vv)
            # 9-neighbor accumulate in PSUM via identity matmuls
            for k4 in range(4):
                nc.tensor.matmul(acc[:, k4 * 512:(k4 + 1) * 512],
                                 id_sb[:], avt[:, k4 * 512:(k4 + 1) * 512],
                                 start=(n == 0), stop=(n == NN - 1))

    def evac_phase(t):
        ch, hh = divmod(t, 2)
        of = of_pool.tile([128, HSLAB], BF16, tag="of", name="of")
        nc.scalar.copy(of[:], accs[t][:])
        nc.sync.dma_start(
            out[ch * 128:(ch + 1) * 128, hh * HSLAB:(hh + 1) * HSLAB],
            of[:])

    # interleaved emission: keeps every engine FIFO stall-free (see header)
    qk_phase(0)
    sm_a_phase(0)
    qk_phase(1)
    sm_b_phase(0)
    av_phase(0)
    sm_a_phase(1)
    qk_phase(2)
    sm_b_phase(1)
    evac_phase(0)
    av_phase(1)
    sm_a_phase(2)
    qk_phase(3)
    sm_b_phase(2)
    evac_phase(1)
    av_phase(2)
    sm_a_phase(3)
    sm_b_phase(3)
    evac_phase(2)
    av_phase(3)
    evac_phase(3)


_CACHE = {}


# --- post-scheduling legalization: this walrus build rejects instructions
# with more sync wait/update commands than the ISA struct has slots; move
# the excess onto standalone EventSemaphore instructions.
WAIT_LIMIT = 1
UPDATE_LIMIT = 1


def _dedup_ldweights(nc):
    f = nc.m.functions[0]
    for blk in f.blocks:
        il = blk.instructions
        keep = []
        last_sig = None
        for ins in il:
            eng = str(getattr(ins, "engine", ""))
            if "PE" in eng:
                if ins.opcode == "Ldweights":
                    si = ins.sync_info
                    clean = si is None or (
                        not list(si.on_wait) and not list(si.on_update))
                    try:
                        sig = repr(ins.ins[0])
                    except Exception:
                        sig = None
                    if sig is not None and sig == last_sig and clean:
                        continue
                    last_sig = sig
                elif ins.opcode not in ("Matmult", "EventSemaphore"):
                    last_sig = None
            keep.append(ins)
        del il[:]
        il.extend(keep)


def _legalize_waits(nc):
    f = nc.m.functions[0]
    for blk in f.blocks:
        il = blk.instructions
        i = 0
        while i < len(il):
            ins = il[i]
            si = ins.sync_info
            if si is None or ins.opcode == "EventSemaphore":
                i += 1
                continue
            waits = list(si.on_wait)
            ups = list(si.on_update)
            changed = False
            if len(waits) > WAIT_LIMIT:
                excess, waits = waits[:-WAIT_LIMIT], waits[-WAIT_LIMIT:]
                for w in excess:
                    ev = mybir.InstEventSemaphore(
                        name=f"lgw-{nc.next_id()}", ins=[], outs=[])
                    ev.engine = ins.engine
                    ev.sync_info = mybir.SyncInfo(on_wait=[w], on_update=[])
                    il.insert(i, ev)
                    i += 1
            post = []
            if len(ups) > UPDATE_LIMIT:
                excess_u, ups = ups[UPDATE_LIMIT:], ups[:UPDATE_LIMIT]
                for u in excess_u:
                    ev = mybir.InstEventSemaphore(
                        name=f"lgu-{nc.next_id()}", ins=[], outs=[])
                    ev.engine = ins.engine
                    ev.sync_info = mybir.SyncInfo(on_wait=[], on_update=[u])
                    post.append(ev)
                changed = True
            if changed or len(list(si.on_wait)) > WAIT_LIMIT:
                ins.sync_info = mybir.SyncInfo(on_wait=waits, on_update=ups)
            for ev in post:
                i += 1
                il.insert(i, ev)
            i += 1


def _get_program():
    if "nc" not in _CACHE:
        nc = bass.Bass("TRN2", target_bir_lowering=False, debug=False)
        with tile.TileContext(nc) as tc:
            with ExitStack() as ctx:
                _build_kernel(nc, ctx, tc)
        if os.environ.get("KERNEL_NO_DEDUP") != "1":
            _dedup_ldweights(nc)
        if os.environ.get("KERNEL_NO_LEGALIZE") != "1":
            _legalize_waits(nc)
        _CACHE["nc"] = nc
    return _CACHE["nc"]


def _shard_inputs(blue_feat, white_feat, q_w, q_b, k_w, k_b, v_w, v_b):
    import ml_dtypes
    blue = np.ascontiguousarray(blue_feat, dtype=np.float16)
    white = np.ascontiguousarray(white_feat, dtype=np.float16)
    wts = {
        "wq": np.ascontiguousarray(np.asarray(q_w, np.float16).T),
        "wk": np.ascontiguousarray(np.asarray(k_w, np.float16).T),
        "wv": np.ascontiguousarray(np.asarray(v_w, np.float16).T),
        "bq": (np.asarray(q_b, np.float32) * SCALE).reshape(2, 128, 1).copy(),
        "bk": np.asarray(k_b, np.float32).reshape(2, 128, 1).copy(),
        "bv": np.asarray(v_b, np.float32).reshape(2, 128, 1).copy(),
        "ident": np.eye(128, dtype=ml_dtypes.bfloat16),
    }
    in_maps = []
    for core in range(NCORES):
        b, qq = divmod(core, QH)
        r0 = qq * NH_LOC * TOK
        xb = blue[b, :, r0:r0 + NH_LOC * TOK, :].reshape(C, SLAB)
        xwp = np.zeros((C, HALO_ROWS * TOK, W), np.float16)
        lo, hi = r0 - TOK, r0 + (NH_LOC + 1) * TOK
        slo, shi = max(lo, 0), min(hi, H)
        xwp[:, slo - lo:shi - lo, :] = white[b, :, slo:shi, :]
        xwp = xwp.reshape(C, PIX_HALO)
        gi = qq * NH_LOC + np.arange(NH_LOC)[:, None, None]
        j = np.arange(NW)[None, :, None]
        di = np.array([o[0] for o in OFFS])[None, None, :]
        dj = np.array([o[1] for o in OFFS])[None, None, :]
        m = ((gi + di >= 0) & (gi + di < NH) &
             (j + dj >= 0) & (j + dj < NW)).astype(np.float32)
        # additive bias: 0 for valid neighbors, -30000 for invalid
        m = (m - 1.0) * (-MASKV)
        # [i, j, n] -> [n, i, j] to match the kernel's n-outer score layout
        m = m.transpose(2, 0, 1).reshape(-1)
        m = np.broadcast_to(m.reshape(1, -1), (128, SNN))
        m = m.astype(ml_dtypes.bfloat16).copy()
        in_maps.append({"xb": np.ascontiguousarray(xb),
                        "xw": np.ascontiguousarray(xwp),
                        "mask": m, **wts})
    return in_maps


def _assemble(results):
    out = np.empty((B, C, H, W), np.float32)
    for core in range(NCORES):
        b, qq = divmod(core, QH)
        r0 = qq * NH_LOC * TOK
        out[b, :, r0:r0 + NH_LOC * TOK, :] = \
            np.asarray(results[core]["out"]).astype(np.float32) \
            .reshape(C, NH_LOC * TOK, W)
    return out


def kernel(blue_feat, white_feat, q_w, q_b, k_w, k_b, v_w, v_b):
    nc = _get_program()
    in_maps = _shard_inputs(blue_feat, white_feat,
                            q_w, q_b, k_w, k_b, v_w, v_b)
    trace = os.environ.get("KERNEL_TRACE") == "1"
    res = run_bass_kernel_spmd(nc, in_maps, core_ids=list(range(NCORES)),
                               trace=trace)
    if trace:
        _CACHE["last_result"] = res
    return _assemble(res.results)


# revision 3
# speedup vs baseline: 1.0100x; 1.0100x over previous
"""CrossModalAttention TRN2 kernel v7: host token-reorder, no e1 stage.

Changes vs v6 (171.1us):
 - Inputs arrive from the host already in (i,u,v,j) token order, so the
   projection matmuls stream CONTIGUOUS columns (v6's strided rearrange
   on the moving operand halved PE throughput and kept the projection
   phase at ~70us).
 - V now uses the same token-order layout + K-style 1-element-shifted
   copy (v1) for dj=+-1 alignment; the per-neighbor e1 v-expansion stage
   (26us of ACT + an extra DVE wait per neighbor) disappears entirely:
   the AV multiply reads the attention weights directly through a merged
   stride-0 (u,v) broadcast dim (innermost j stays stride-1 -> DVE 2x).
 - Output returned in token order; the host un-permutes.
 - k1 built by DVE during projection (fills its pre-attention idle
   window); v1 by chunked SBUF->SBUF DMA (needed much later).

Changes vs v5 (197.8us, wrong results):
 - GpSimd dropped entirely: its tensor_copy mishandled the stride-0
   broadcast AP (wrong e1 values) AND any GP activity port-blocks
   concurrent DVE tensor_tensor ops down to 1x.
 - Projection evacuation split ACT (low half) / DVE scalar_tensor_tensor
   (high half, different PSUM banks) to keep the PE duty cycle high
   enough to hold HAM at K=8/8.

Changes vs v4 (kernel.py baseline, 196.7us):
 - QK tree-reduce (47us DVE) replaced by PE identity-matmul PSUM
   accumulation over the 16 token pixels (16 mms x 128 cols per
   neighbor).  PE holds ONE stationary identity weight through the whole
   attention phase (no Ldweights churn) and never idles long enough to
   re-throttle (HAM stays at K=8/8).
 - Mask becomes an ADDITIVE -30000 bias accumulated as a 17th matmul
   into the score PSUM; exp underflows masked slots to exactly 0 (the
   separate em = e*mask DVE multiply disappears).
 - Z = sum_n e via 9 more identity matmuls (PSUM accumulate), and
   1/Z = exp(-ln Z) on ACT (Exp+Ln+Identity+Copy live in one table
   set) -- the expensive DVE reciprocal disappears.
 - AV multiply fused 4->1 per neighbor via a stride-0 u-broadcast AP on
   the attention weights (innermost stays stride-1 so DVE keeps 2x).
 - Work split into 4 slabs (2 channel-halves x 2 token-row-halves) so
   score PSUM (3 banks) + z (1 bank) + AV accumulator (4 banks) fit the
   8 PSUM banks, and the slab phases interleave across DVE/PE/ACT/GP.
 - e1 (attention-weight v-expansion) alternates GpSimd / ACT.
"""

import os
from contextlib import ExitStack

import numpy as np

import concourse.bass as bass
import concourse.mybir as mybir
import concourse.tile as tile
from concourse.bass_utils import run_bass_kernel_spmd

B, C, H, W = 2, 256, 128, 128
TOK = 4
NH, NW = H // TOK, W // TOK          # 32, 32
T2 = TOK * TOK
SCALE = float((C // T2) ** -0.5)
NCORES = 8
QH = 4
NH_LOC = NH // QH                    # 8 token rows / core
ROWSZ = TOK * W                      # 512 px per token row
SLAB = NH_LOC * ROWSZ                # 4096 px per channel-half
HALO_ROWS = NH_LOC + 2
PIX_HALO = HALO_ROWS * ROWSZ         # 5120
NIJ = NH_LOC * NW                    # 256 tokens per core
OFFS = [(di, dj) for di in (-1, 0, 1) for dj in (-1, 0, 1)]
NN = len(OFFS)                       # 9
SNN = NN * NIJ                       # 2304 score slots per channel-half

# slab = (channel-half, token-row-half): 4 token rows each
IH = NH_LOC // 2                     # 4 token rows per slab
HSLAB = IH * ROWSZ                   # 2048 px per slab
HNIJ = IH * NW                       # 128 tokens per slab
HSNN = NN * HNIJ                     # 1152 score slots per slab
MASKV = -30000.0                     # additive mask: exp(s-30000) == 0

F32 = mybir.dt.float32
FP16 = mybir.dt.float16
BF16 = mybir.dt.bfloat16
AF = mybir.ActivationFunctionType

KPAD = 2                             # even pad: dj=0 reads stay 4B-aligned
KSZ = KPAD + PIX_HALO + KPAD
K1SZ = PIX_HALO + 2                  # K/V shifted by one elem: dj=+-1 aligned


def _build_kernel(nc: bass.Bass, ctx: ExitStack, tc: "tile.TileContext"):
    xb = nc.dram_tensor("xb", [C, SLAB], FP16, kind="ExternalInput").ap()
    xw = nc.dram_tensor("xw", [C, PIX_HALO], FP16, kind="ExternalInput").ap()
    wq = nc.dram_tensor("wq", [C, C], FP16, kind="ExternalInput").ap()
    wk = nc.dram_tensor("wk", [C, C], FP16, kind="ExternalInput").ap()
    wv = nc.dram_tensor("wv", [C, C], FP16, kind="ExternalInput").ap()
    bq = nc.dram_tensor("bq", [2, 128, 1], F32, kind="ExternalInput").ap()
    bk = nc.dram_tensor("bk", [2, 128, 1], F32, kind="ExternalInput").ap()
    bv = nc.dram_tensor("bv", [2, 128, 1], F32, kind="ExternalInput").ap()
    mask = nc.dram_tensor("mask", [128, SNN], BF16, kind="ExternalInput").ap()
    ident = nc.dram_tensor("ident", [128, 128], BF16,
                           kind="ExternalInput").ap()
    out = nc.dram_tensor("out", [C, SLAB], BF16, kind="ExternalOutput").ap()

    const_pool = ctx.enter_context(tc.tile_pool(name="const", bufs=1))

    # ---- persistent constants
    w_sb = {}
    for name, wd in (("q", wq), ("k", wk), ("v", wv)):
        for ci in range(2):
            t = const_pool.tile([128, C], FP16, tag=f"w{name}{ci}",
                                name=f"w{name}{ci}")
            nc.sync.dma_start(t[:], wd[ci * 128:(ci + 1) * 128, :])
            w_sb[name, ci] = t
    b_sb = {}
    for name, bd in (("q", bq), ("k", bk), ("v", bv)):
        for co in range(2):
            t = const_pool.tile([128, 1], F32, tag=f"b{name}{co}",
                                name=f"b{name}{co}")
            nc.sync.dma_start(t[:], bd[co])
            b_sb[name, co] = t
    mask_sb = const_pool.tile([128, SNN], BF16, tag="mask", name="mask")
    nc.sync.dma_start(mask_sb[:], mask[:])
    id_sb = const_pool.tile([128, 128], BF16, tag="ident", name="ident")
    nc.sync.dma_start(id_sb[:], ident[:])

    # ACT/DVE warm-ups: cover the bias/mask DMAs on their consumer engines
    # so the real consumers carry at most one sync wait (walrus limit).
    scratch = const_pool.tile([128, 16], F32, tag="scratch", name="scratch")
    for wi, name in enumerate(("q", "k", "v")):
        for co in range(2):
            nc.scalar.activation(scratch[:, wi * 2 + co:wi * 2 + co + 1],
                                 b_sb[name, co][:], AF.Identity,
                                 bias=b_sb[name, co][:])
            # DVE also consumes the biases (STT proj evacs) -- warm them
            nc.vector.tensor_copy(scratch[:, 8 + wi * 2 + co:
                                          9 + wi * 2 + co],
                                  b_sb[name, co][:])
    nc.vector.tensor_copy(scratch[:, 6:7], mask_sb[:, 0:1])

    # attention SBUF pools (allocated before x so x can be scoped/freed)
    qkv_pool = ctx.enter_context(tc.tile_pool(name="qkv", bufs=1))
    e_pool = ctx.enter_context(tc.tile_pool(name="e", bufs=2))
    an_pool = ctx.enter_context(tc.tile_pool(name="an", bufs=2))
    ln_pool = ctx.enter_context(tc.tile_pool(name="ln", bufs=2))
    zr_pool = ctx.enter_context(tc.tile_pool(name="zr", bufs=2))
    prod_pool = ctx.enter_context(tc.tile_pool(name="prod", bufs=3))
    avt_pool = ctx.enter_context(tc.tile_pool(name="avt", bufs=3))
    of_pool = ctx.enter_context(tc.tile_pool(name="of", bufs=2))

    q_sb = [qkv_pool.tile([128, SLAB], BF16, tag=f"q{c}", name=f"q{c}")
            for c in range(2)]
    k_sb = [qkv_pool.tile([128, KSZ], BF16, tag=f"k{c}", name=f"k{c}")
            for c in range(2)]
    k1_sb = [qkv_pool.tile([128, K1SZ], BF16, tag=f"k1{c}", name=f"k1{c}")
             for c in range(2)]
    v_sb = [qkv_pool.tile([128, KSZ], BF16, tag=f"v{c}", name=f"v{c}")
            for c in range(2)]
    v1_sb = [qkv_pool.tile([128, K1SZ], BF16, tag=f"v1{c}", name=f"v1{c}")
             for c in range(2)]
    for t in k_sb + v_sb:
        nc.vector.memset(t[:, 0:KPAD], 0.0)
        nc.vector.memset(t[:, KPAD + PIX_HALO:], 0.0)
    for t in k1_sb + v1_sb:
        # chunked shift copies cover y in [1, 5121); the two edge
        # elements mirror the pad zeros
        nc.vector.memset(t[:, 0:1], 0.0)
        nc.vector.memset(t[:, K1SZ - 1:], 0.0)

    with tc.tile_pool(name="ps", bufs=2, space="PSUM") as ps_pool:
        # PE warm-up: cover weight/ident/mask DMAs on PE's clock.
        warm_ps = ps_pool.tile([128, 2048], F32, tag="ps", name="warm")
        for name in ("q", "k", "v"):
            for ci in range(2):
                nc.tensor.matmul(warm_ps[0:1, 0:1], w_sb[name, ci][:, 0:1],
                                 w_sb[name, ci][:, 0:1],
                                 start=True, stop=True)
        nc.tensor.matmul(warm_ps[0:1, 0:1], id_sb[:, 0:1], id_sb[:, 0:1],
                         start=True, stop=True)
        nc.tensor.matmul(warm_ps[0:1, 0:1], id_sb[:, 0:1], mask_sb[:, 0:1],
                         start=True, stop=True)

        # ---- projections (PE matmul, evacuation via ACT)
        with tc.tile_pool(name="x", bufs=1) as x_pool:
            xb_sb = [x_pool.tile([128, SLAB], FP16, tag=f"xb{ci}",
                                 name=f"xb{ci}") for ci in range(2)]
            xw_sb = [x_pool.tile([128, PIX_HALO], FP16, tag=f"xw{ci}",
                                 name=f"xw{ci}") for ci in range(2)]
            for ci in range(2):
                rows = slice(ci * 128, (ci + 1) * 128)
                for c0 in range(0, SLAB, 1024):
                    nc.sync.dma_start(xb_sb[ci][:, c0:c0 + 1024],
                                      xb[rows, c0:c0 + 1024])
            for ci in range(2):
                rows = slice(ci * 128, (ci + 1) * 128)
                for c0 in range(0, PIX_HALO, 1024):
                    nc.sync.dma_start(xw_sb[ci][:, c0:c0 + 1024],
                                      xw[rows, c0:c0 + 1024])
                # dummy matmuls put every x-DMA queue on PE's clock
                for pt in range(SLAB // 512):
                    nc.tensor.matmul(warm_ps[:, 0:64],
                                     xb_sb[ci][:, pt * 512:pt * 512 + 128],
                                     xb_sb[ci][:, pt * 512:pt * 512 + 64],
                                     start=True, stop=True)
                for pt in range(PIX_HALO // 512):
                    nc.tensor.matmul(warm_ps[:, 0:64],
                                     xw_sb[ci][:, pt * 512:pt * 512 + 128],
                                     xw_sb[ci][:, pt * 512:pt * 512 + 64],
                                     start=True, stop=True)

            def project(name, co, src, dst, pix, pad, scale,
                        sh_dst=None, sh_dma=False):
                for c0 in range(0, pix, 2048):
                    cw = min(2048, pix - c0)
                    ps = ps_pool.tile([128, 2048], F32, tag="ps")
                    for ci in range(2):
                        for s0 in range(0, cw, 512):
                            sl = slice(c0 + s0, c0 + s0 + 512)
                            nc.tensor.matmul(
                                ps[:, s0:s0 + 512],
                                w_sb[name, ci][:, co * 128:(co + 1) * 128],
                                src[ci][:, sl],
                                start=(ci == 0), stop=(ci == 1))
                    if sh_dst is not None:
                        # full ACT evac, then build the 1-elem-shifted
                        # copy of this chunk (sh[y] = dst[y+1]) on DVE
                        # (k1: needed early) or DMA (v1: needed late)
                        nc.scalar.activation(
                            dst[co][:, pad + c0:pad + c0 + cw],
                            ps[:, 0:cw], AF.Identity,
                            bias=b_sb[name, co][:], scale=scale)
                        if sh_dma:
                            nc.sync.dma_start(
                                sh_dst[co][:, c0 + 1:c0 + 1 + cw],
                                dst[co][:, c0 + 2:c0 + 2 + cw])
                        else:
                            nc.vector.tensor_copy(
                                sh_dst[co][:, c0 + 1:c0 + 1 + cw],
                                dst[co][:, c0 + 2:c0 + 2 + cw])
                    else:
                        # split evac: ACT low half (banks 0-1), DVE STT
                        # high half (banks 2-3) -- keeps the chunk cadence
                        # at the PE's pace so HAM stays warm
                        h1 = cw // 2
                        nc.scalar.activation(
                            dst[co][:, pad + c0:pad + c0 + h1],
                            ps[:, 0:h1], AF.Identity,
                            bias=b_sb[name, co][:], scale=scale)
                        nc.vector.scalar_tensor_tensor(
                            dst[co][:, pad + c0 + h1:pad + c0 + cw],
                            ps[:, h1:cw], float(scale),
                            b_sb[name, co][:].broadcast_to((128, cw - h1)),
                            mybir.AluOpType.mult, mybir.AluOpType.add)

            # channel-half 0 first so attention slabs 0/1 start early
            for co in range(2):
                project("q", co, xb_sb, q_sb, SLAB, 0, SCALE)
                # K/V shifted one element left: dj=+-1 reads hit K1/V1 at
                # even (4B-aligned) offsets, keeping DVE 2x
                project("k", co, xw_sb, k_sb, PIX_HALO, KPAD, 1.0,
                        sh_dst=k1_sb)
                project("v", co, xw_sb, v_sb, PIX_HALO, KPAD, 1.0,
                        sh_dst=v1_sb, sh_dma=True)

    # ---- attention PSUM pools (after proj pool closed): 3 + 1 + 4 banks
    s_pool = ctx.enter_context(tc.tile_pool(name="sps", bufs=1,
                                            space="PSUM"))
    z_pool = ctx.enter_context(tc.tile_pool(name="zps", bufs=1,
                                            space="PSUM"))
    av_pool = ctx.enter_context(tc.tile_pool(name="avps", bufs=1,
                                             space="PSUM"))

    mask_v = mask_sb[:].rearrange("p (n i j) -> p n i j", n=NN, i=NH_LOC)
    s_ts = {}
    e_ts = {}
    an_ts = {}
    accs = {}

    def qk_phase(t):
        ch, hh = divmod(t, 2)
        # [128, 1536] f32 = 3 banks; scores live in [0:1152]
        s_t = s_pool.tile([128, 3 * 512], F32, tag="s", name=f"s{t}")
        s_ts[t] = s_t
        q_sl = q_sb[ch][:, hh * HSLAB:(hh + 1) * HSLAB]
        for n in (1, 4, 7, 0, 2, 3, 5, 6, 8):
            di, dj = OFFS[n]
            base = (hh * IH + di + 1) * ROWSZ
            if dj == 0:
                ksrc, koff = k_sb[ch], KPAD + base
            else:
                # K1[y] = K[y+1] -> K[base + dj + t] = K1[base + dj + 1 + t]
                ksrc, koff = k1_sb[ch], base + dj + 1
            prod = prod_pool.tile([128, HSLAB], BF16, tag="prod",
                                  name="prod")
            nc.vector.tensor_mul(prod[:], q_sl,
                                 ksrc[:, koff:koff + HSLAB])
            # PE: sum over the 16 token pixels (u,v) + additive mask bias
            pv = prod[:].rearrange("p (i u v j) -> p u v i j",
                                   i=IH, u=TOK, v=TOK)
            sreg = s_t[:, n * HNIJ:(n + 1) * HNIJ]
            for u in range(TOK):
                for v in range(TOK):
                    nc.tensor.matmul(sreg, id_sb[:], pv[:, u, v],
                                     start=(u == 0 and v == 0), stop=False)
            mv = mask_v[:, n, hh * IH:(hh + 1) * IH]
            nc.tensor.matmul(sreg, id_sb[:], mv, start=False, stop=True)

    def sm_a_phase(t):
        # everything that touches the score PSUM tile: exp, Z-accum, ln
        s_t = s_ts[t]
        e_t = e_pool.tile([128, HSNN], BF16, tag="e", name=f"e{t}")
        e_ts[t] = e_t
        nc.scalar.activation(e_t[:], s_t[:, 0:HSNN], AF.Exp)
        z_t = z_pool.tile([128, 512], F32, tag="z", name=f"z{t}")
        ev = e_t[:].rearrange("p (n ij) -> p n ij", n=NN)
        for n in range(NN):
            nc.tensor.matmul(z_t[:, 0:HNIJ], id_sb[:], ev[:, n],
                             start=(n == 0), stop=(n == NN - 1))
        lnz = ln_pool.tile([128, HNIJ], F32, tag="lnz", name=f"lnz{t}")
        nc.scalar.activation(lnz[:], z_t[:, 0:HNIJ], AF.Ln)
        s_ts[t] = lnz

    def sm_b_phase(t):
        lnz = s_ts[t]
        zrb = zr_pool.tile([128, HNIJ], BF16, tag="zrb", name=f"zrb{t}")
        nc.scalar.activation(zrb[:], lnz[:], AF.Exp, scale=-1.0)
        an_t = an_pool.tile([128, HSNN], BF16, tag="an", name=f"an{t}")
        ev = e_ts[t][:].rearrange("p (n ij) -> p n ij", n=NN)
        anv = an_t[:].rearrange("p (n ij) -> p n ij", n=NN)
        zb = zrb[:].unsqueeze(1).broadcast_to((128, NN, HNIJ))
        nc.vector.tensor_mul(anv, ev, zb)
        an_ts[t] = an_t

    def av_phase(t):
        ch, hh = divmod(t, 2)
        an_t = an_ts[t]
        acc = av_pool.tile([128, HSLAB], F32, tag="av", name=f"acc{t}")
        accs[t] = acc
        for n, (di, dj) in enumerate(OFFS):
            # fused AV multiply: the attention weight broadcasts over the
            # merged (u,v) dim via stride-0 (innermost j stays stride-1,
            # so the DVE keeps 2x); no expansion stage needed
            base = (hh * IH + di + 1) * ROWSZ
            if dj == 0:
                vsrc, voff = v_sb[ch], KPAD + base
            else:
                vsrc, voff = v1_sb[ch], base + dj + 1
            avt = avt_pool.tile([128, HSLAB], BF16, tag="avt", name="avt")
            av_v = avt[:].rearrange("p (i uv j) -> p i uv j", i=IH, uv=T2)
            anb = an_t[:, n * HNIJ:(n + 1) * HNIJ].rearrange(
                "p (i j) -> p i j", i=IH) \
                .unsqueeze(2).broadcast_to((128, IH, T2, NW))
            vv = vsrc[:, voff:voff + HSLAB].rearrange(
                "p (i uv j) -> p i uv j", i=IH, uv=T2)
            nc.vector.tensor_mul(av_v, anb, vv)
            # 9-neighbor accumulate in PSUM via identity matmuls
            for k4 in range(4):
                nc.tensor.matmul(acc[:, k4 * 512:(k4 + 1) * 512],
                                 id_sb[:], avt[:, k4 * 512:(k4 + 1) * 512],
                                 start=(n == 0), stop=(n == NN - 1))

    def evac_phase(t):
        ch, hh = divmod(t, 2)
        of = of_pool.tile([128, HSLAB], BF16, tag="of", name="of")
        nc.scalar.copy(of[:], accs[t][:])
        nc.sync.dma_start(
            out[ch * 128:(ch + 1) * 128, hh * HSLAB:(hh + 1) * HSLAB],
            of[:])

    # interleaved emission: keeps every engine FIFO stall-free (see header)
    qk_phase(0)
    sm_a_phase(0)
    qk_phase(1)
    sm_b_phase(0)
    av_phase(0)
    sm_a_phase(1)
    qk_phase(2)
    sm_b_phase(1)
    evac_phase(0)
    av_phase(1)
    sm_a_phase(2)
    qk_phase(3)
    sm_b_phase(2)
    evac_phase(1)
    av_phase(2)
    sm_a_phase(3)
    sm_b_phase(3)
    evac_phase(2)
    av_phase(3)
    evac_phase(3)


_CACHE = {}


# --- post-scheduling legalization: this walrus build rejects instructions
# with more sync wait/update commands than the ISA struct has slots; move
# the excess onto standalone EventSemaphore instructions.
WAIT_LIMIT = 1
UPDATE_LIMIT = 1


def _dedup_ldweights(nc):
    f = nc.m.functions[0]
    for blk in f.blocks:
        il = blk.instructions
        keep = []
        last_sig = None
        for ins in il:
            eng = str(getattr(ins, "engine", ""))
            if "PE" in eng:
                if ins.opcode == "Ldweights":
                    si = ins.sync_info
                    clean = si is None or (
                        not list(si.on_wait) and not list(si.on_update))
                    try:
                        sig = repr(ins.ins[0])
                    except Exception:
                        sig = None
                    if sig is not None and sig == last_sig and clean:
                        continue
                    last_sig = sig
                elif ins.opcode not in ("Matmult", "EventSemaphore"):
                    last_sig = None
            keep.append(ins)
        del il[:]
        il.extend(keep)


def _legalize_waits(nc):
    f = nc.m.functions[0]
    for blk in f.blocks:
        il = blk.instructions
        i = 0
        while i < len(il):
            ins = il[i]
            si = ins.sync_info
            if si is None or ins.opcode == "EventSemaphore":
                i += 1
                continue
            waits = list(si.on_wait)
            ups = list(si.on_update)
            changed = False
            if len(waits) > WAIT_LIMIT:
                excess, waits = waits[:-WAIT_LIMIT], waits[-WAIT_LIMIT:]
                for w in excess:
                    ev = mybir.InstEventSemaphore(
                        name=f"lgw-{nc.next_id()}", ins=[], outs=[])
                    ev.engine = ins.engine
                    ev.sync_info = mybir.SyncInfo(on_wait=[w], on_update=[])
                    il.insert(i, ev)
                    i += 1
            post = []
            if len(ups) > UPDATE_LIMIT:
                excess_u, ups = ups[UPDATE_LIMIT:], ups[:UPDATE_LIMIT]
                for u in excess_u:
                    ev = mybir.InstEventSemaphore(
                        name=f"lgu-{nc.next_id()}", ins=[], outs=[])
                    ev.engine = ins.engine
                    ev.sync_info = mybir.SyncInfo(on_wait=[], on_update=[u])
                    post.append(ev)
                changed = True
            if changed or len(list(si.on_wait)) > WAIT_LIMIT:
                ins.sync_info = mybir.SyncInfo(on_wait=waits, on_update=ups)
            for ev in post:
                i += 1
                il.insert(i, ev)
            i += 1


def _get_program():
    if "nc" not in _CACHE:
        nc = bass.Bass("TRN2", target_bir_lowering=False, debug=False)
        with tile.TileContext(nc) as tc:
            with ExitStack() as ctx:
                _build_kernel(nc, ctx, tc)
        if os.environ.get("KERNEL_NO_DEDUP") != "1":
            _dedup_ldweights(nc)
        if os.environ.get("KERNEL_NO_LEGALIZE") != "1":
            _legalize_waits(nc)
        _CACHE["nc"] = nc
    return _CACHE["nc"]


def _tokord(x_pix):
    # [C, P, W] pixel rows -> [C, P*W] in (i, u, v, j) token order
    Cc, P, _ = x_pix.shape
    rows = P // TOK
    t = x_pix.reshape(Cc, rows, TOK, NW, TOK)        # c, i, u, j, v
    return np.ascontiguousarray(t.transpose(0, 1, 2, 4, 3)) \
        .reshape(Cc, rows * ROWSZ)


def _shard_inputs(blue_feat, white_feat, q_w, q_b, k_w, k_b, v_w, v_b):
    import ml_dtypes
    blue = np.ascontiguousarray(blue_feat, dtype=np.float16)
    white = np.ascontiguousarray(white_feat, dtype=np.float16)
    wts = {
        "wq": np.ascontiguousarray(np.asarray(q_w, np.float16).T),
        "wk": np.ascontiguousarray(np.asarray(k_w, np.float16).T),
        "wv": np.ascontiguousarray(np.asarray(v_w, np.float16).T),
        "bq": (np.asarray(q_b, np.float32) * SCALE).reshape(2, 128, 1).copy(),
        "bk": np.asarray(k_b, np.float32).reshape(2, 128, 1).copy(),
        "bv": np.asarray(v_b, np.float32).reshape(2, 128, 1).copy(),
        "ident": np.eye(128, dtype=ml_dtypes.bfloat16),
    }
    in_maps = []
    for core in range(NCORES):
        b, qq = divmod(core, QH)
        r0 = qq * NH_LOC * TOK
        xb = _tokord(blue[b, :, r0:r0 + NH_LOC * TOK, :])
        xwp = np.zeros((C, HALO_ROWS * TOK, W), np.float16)
        lo, hi = r0 - TOK, r0 + (NH_LOC + 1) * TOK
        slo, shi = max(lo, 0), min(hi, H)
        xwp[:, slo - lo:shi - lo, :] = white[b, :, slo:shi, :]
        xwp = _tokord(xwp)
        gi = qq * NH_LOC + np.arange(NH_LOC)[:, None, None]
        j = np.arange(NW)[None, :, None]
        di = np.array([o[0] for o in OFFS])[None, None, :]
        dj = np.array([o[1] for o in OFFS])[None, None, :]
        m = ((gi + di >= 0) & (gi + di < NH) &
             (j + dj >= 0) & (j + dj < NW)).astype(np.float32)
        # additive bias: 0 for valid neighbors, -30000 for invalid
        m = (m - 1.0) * (-MASKV)
        # [i, j, n] -> [n, i, j] to match the kernel's n-outer score layout
        m = m.transpose(2, 0, 1).reshape(-1)
        m = np.broadcast_to(m.reshape(1, -1), (128, SNN))
        m = m.astype(ml_dtypes.bfloat16).copy()
        in_maps.append({"xb": np.ascontiguousarray(xb),
                        "xw": np.ascontiguousarray(xwp),
                        "mask": m, **wts})
    return in_maps


def _assemble(results):
    out = np.empty((B, C, H, W), np.float32)
    for core in range(NCORES):
        b, qq = divmod(core, QH)
        r0 = qq * NH_LOC * TOK
        oc = np.asarray(results[core]["out"]).astype(np.float32) \
            .reshape(C, NH_LOC, TOK, TOK, NW)        # c, i, u, v, j
        out[b, :, r0:r0 + NH_LOC * TOK, :] = \
            oc.transpose(0, 1, 2, 4, 3).reshape(C, NH_LOC * TOK, W)
    return out


def kernel(blue_feat, white_feat, q_w, q_b, k_w, k_b, v_w, v_b):
    nc = _get_program()
    in_maps = _shard_inputs(blue_feat, white_feat,
                            q_w, q_b, k_w, k_b, v_w, v_b)
    trace = os.environ.get("KERNEL_TRACE") == "1"
    res = run_bass_kernel_spmd(nc, in_maps, core_ids=list(range(NCORES)),
                               trace=trace)
    if trace:
        _CACHE["last_result"] = res
    return _assemble(res.results)
